# revision 32
# baseline (speedup 1.0000x reference)
"""Trainium2 Bass kernel: fp8 DoubleRow Euler integrator (8-core data-parallel).

Reference per step: uv = v@U.T; c = (uv*uv)@W.T; x += dt*v (old v);
v += dt*(force - c); x wrapped into [-pi,pi).

Design (per core, batch shard BS=512, transposed on-chip layout [feat, b];
default config: a_mode=fp8, n_streams=2, sq_span=2, uv_bufs=2, xsum_skip0):

- v state lives in PSUM: psum_v = (v - t*dt*force)/alpha (fp32, one bank
  per d-half). Phase B accumulates -dt*c into it directly, so the only
  per-step DVE state op is the operand cast. The deterministic force ramp
  t*dt*force is never computed on device: the cast re-adds it from a
  host-precomputed bf16 table (v8 = (alpha*pv)*SV8 + SV8*(t+1)*dt*force),
  and the exact fp32 ramp is folded into the x0/vo paths on host.
- Phase A: uv = U@v via fp8 DoubleRow matmuls: one NON-accumulating DR per
  h-tile contracts all of K=256 (both d-halves in the DR pair slots).
  Accumulating matmuls run at ~half the column rate of start=True matmuls
  on HW (PSUM read-modify-write; NOT modeled by TimelineSim/CoreSim).
- Squares are SCALE-FREE: sqrt(QS/r_k) is folded into the per-step u8
  dither copies on host, so sq8 = e4m3(Square(uv)) with scale=1 — a
  single op on ACT (and would be a single tensor_tensor on DVE, except
  DVE ops may read only ONE PSUM operand, so a DVE square of PSUM uv
  needs 2 ops and is never worth it).
- Phase B: 4 accumulating DR matmuls per d-half per step (K_eff=256 each)
  into psum_v.
- Quantization error control (rel_err ~6.1e-3 vs the 2e-2 gate):
  * W8/U8 are scale-dithered: 16 fp8 copies at r_k = 2^(k/16) (u8 at
    sqrt(QS/r_k)/SV8), one per step. This turns systematic fp8
    weight-quantization error into a random walk across the 16 steps.
  * x is NOT integrated from quantized v: xsum (SBUF fp32) accumulates
    alpha*psum_v exactly on DVE each step (the x gate is dominated by
    +-2pi wrap flips of elements near the torus boundary; ~1e-4 RMS of
    extra x noise costs ~10s of flips at 6e-3 rel each — any fp8-based
    x path fails). The t=0 sample is skipped: DT*v0 is folded into
    x0pi exactly on host (xsum_skip0).
  * v0/alpha is injected into psum_v via two bf16 identity matmuls
    (hi/lo bf16 split, error ~2^-16); the final wrap into [-pi,pi) is a
    single +-2pi range reduction (unwrapped x stays in (-2pi, 4pi)).
- n_streams=2 splits the batch into two independent 256-column streams,
  staggering the serial per-step chain (cast -> A -> square -> B -> cast)
  of one stream against engine work of the other; sq_span=2 batches the
  squares of two h-pairs into one ACT op over a 2-bank uv tile.

HW budget (measured via For_i loop marginal, 50 vs 1050 reps): 16-step
body ~77us = 4.8us/step. ACT is the binding engine: squares alone
(sq_only variant) measure 68us = 4.26us/step (H*BS/128 = 4096 lanes-cols
at ~0.83ns + op inits, hard floor ~3.4us/step), so the step runs at ~90%
ACT occupancy. DVE (xsum+cast, ~2.6us/step) and PE (~2.6us/step with the
HW 2x accumulation penalty) have slack, but no measured redistribution
beats this point:
- DVE square offload: illegal as 1 op (two PSUM reads), 2x cost as 2 ops.
- force-inject via identity matmuls (to free the cast): +1.7us/step PE
  on HW (accum penalty) -> 88.7us body. Reverted.
- sq_span=4 (fewer ACT inits): PSUM forces uv_bufs=1 -> serializes
  streams -> 166us. Asym groups (3,1): same PSUM wall in sim.
- uv_bufs=3, sq_bufs=8, merged [P,2,BS] pv ops, staggered_reset For_i,
  32-step bodies, n_streams=4, cast-on-ACT: all neutral-to-worse on HW.
- phase-major emission (both streams' A+squares before any B, uv_bufs=3):
  86-88us — stretches each stream's serial loop; stream-major is right.
- PE DVFS filler matmuls (keep the p-state hot through square waits):
  83-95us, monotonically worse with filler count — in-order PE queue
  delay on B dominates any clock-ramp gain.
- bf16 uv PSUM tiles (would halve uv banks, enable sq_span=4 + bufs=2):
  TRN2 matmul output must be fp32; 16-bit PSUM out is TRN3+.
- xsum_defer (emit xsum(t+1,s) right after cast(t,s), ahead of the other
  stream's cast in the DVE queue): 82us vs 78 base. cast_cols=128: 85us.
  sq_bufs=3: neutral. All interleaved-measured in one clean window.
- hoisting both streams' xsums to the step start: 104us (!) — xsum(s1,t)
  waits on B(t-1,s1), which lands HALF A STEP late (the stagger), so its
  sem wait blocks cast(s0) in the in-order DVE queue. The stream-major
  emission [xs(s0), cast(s0), xs(s1), cast(s1)] is exactly the staggered
  readiness order — any permutation loses 4-27us.
- Asymmetric stream column-splits are zero-sum: the two step-boundary
  gaps (each stream's serial-chain latency minus the other stream's ACT
  cover) sum to a split-independent constant set by fixed sem-hop
  latency (~0.4us/stream) — the residual ~0.5us/step over the ACT floor
  is structural.
- HW timing drifts: one ~20% slow episode observed across all variants
  for ~10 min; interleave A/B measurements and re-measure the base.
"""

import contextlib

import numpy as np
import ml_dtypes

import concourse.bacc as bacc
import concourse.mybir as mybir
import concourse.tile as tile
from concourse.bass_utils import run_bass_kernel_spmd

F32 = mybir.dt.float32
BF16 = mybir.dt.bfloat16
FP8 = mybir.dt.float8e4
ALU = mybir.AluOpType
ACTF = mybir.ActivationFunctionType
DR = mybir.MatmulPerfMode.DoubleRow

NPF8 = ml_dtypes.float8_e4m3
NPBF = ml_dtypes.bfloat16

N_CORES = 8
B = 4096
D = 256
H = 1024
P = 128
BS = B // N_CORES          # 512
ND = D // P                # 2 d-halves
NH = H // P                # 8 h-tiles
NPAIR = NH // 2            # 4 h-pairs

DT = np.float32(0.01)
PI = float(np.pi)
TWO_PI = float(2.0 * np.pi)

KW = 16384.0
QS = 4.0
ALPHA = 1.0 / (KW * QS)
SU = 32.0           # fp8 phase-A stationary scale (bf16-mode legacy)
SV = 16.0           # fp8 phase-A moving scale (bf16-mode legacy)
SV8 = 1.0           # scale-free-square fp8 moving scale
GF = 32.0           # force hi-slot stationary scale
N_DITHER = 16

_PROGRAM_CACHE: dict = {}
A_MODE = "fp8"    # "bf16" | "fp8" - selects the phase-A implementation


def _r_k(k):
    return 2.0 ** ((k % N_DITHER) / N_DITHER)


def _q8(x):
    x = np.clip(np.asarray(x, np.float32), -240.0, 240.0)
    return x.astype(NPF8)


def _build(steps: int, loop_reps: int | None = None, variant: str = "full",
           sq_dve_pairs: int = 0, uv_bufs: int = 3, sq_bufs: int = 4,
           a_mode: str = "bf16", cast_cols: int = 0, n_streams: int = 1,
           sq_dve_units: int = 0, sq_span: int = 1, sq_sliver: int = 0,
           cast_eng: str = "dve", sq_groups: tuple | None = None,
           uv2_bufs: int = 2, loop_stagger: bool = False,
           xsum_skip0: bool = False, xsum_wide: bool = False,
           uv_bf16: bool = False, phase_major: bool = False,
           pe_fill: int = 0, xsum_defer: bool = False):
    """variant: full | mm_only (no squares: B consumes stale sq tiles) |
    sq_only (A+squares, no B/casts).
    a_mode: bf16 (plain bf16 phase A, force folded into casts) |
            fp8 (DoubleRow fp8 phase A, force via fp8 DR pair in phase B).
    cast_cols: if >0, split each cast into column chunks of this width."""
    do_sq = variant in ("full", "sq_only", "no_xsum", "no_cast", "no_xc")
    do_b = variant in ("full", "mm_only", "no_xsum", "no_cast", "no_xc")
    do_xsum = variant not in ("no_xsum", "no_xc")
    do_cast = variant not in ("no_cast", "no_xc")
    fp8a = a_mode == "fp8"
    nc = bacc.Bacc(None, target_bir_lowering=False)

    vi0_d = nc.dram_tensor("vi0", [D, BS], BF16, kind="ExternalInput")
    vi1_d = nc.dram_tensor("vi1", [D, BS], BF16, kind="ExternalInput")
    x0_d = nc.dram_tensor("x0pi", [D, BS], F32, kind="ExternalInput")
    ff_d = nc.dram_tensor("ffin", [D, BS], F32, kind="ExternalInput")
    w8_d = nc.dram_tensor("w8d", [N_DITHER * P, NH * D], FP8, kind="ExternalInput")
    idb_d = nc.dram_tensor("idb", [P, P], BF16, kind="ExternalInput")
    tdtf_d = nc.dram_tensor("tdtf", [steps * D, BS], BF16, kind="ExternalInput")
    if fp8a:
        u8_d = nc.dram_tensor("u8d", [N_DITHER * P, 2 * H], FP8, kind="ExternalInput")
        v80_d = nc.dram_tensor("v80", [P, 2 * BS], FP8, kind="ExternalInput")
    else:
        utb_d = nc.dram_tensor("utb", [D, H], BF16, kind="ExternalInput")
        vb0_d = nc.dram_tensor("vb0", [D, BS], BF16, kind="ExternalInput")
    xo_d = nc.dram_tensor("xo", [D, BS], F32, kind="ExternalOutput")
    vo_d = nc.dram_tensor("vo", [D, BS], F32, kind="ExternalOutput")

    nk = min(N_DITHER, steps) if steps > 0 else 1

    with tile.TileContext(nc) as tc:
        with (
            tc.tile_pool(name="state", bufs=1) as state,
            tc.tile_pool(name="sq", bufs=sq_bufs) as sqp,
            tc.tile_pool(name="tmp", bufs=4) as tmp,
            tc.tile_pool(name="psuv", bufs=uv_bufs, space="PSUM") as ps_uv,
            tc.tile_pool(name="psuv2", bufs=uv2_bufs, space="PSUM") as ps_uv2,
            tc.tile_pool(name="psst", bufs=1, space="PSUM") as ps_state,
            tc.tile_pool(name="psfill", bufs=1, space="PSUM") as ps_fill,
        ):
            v_bf = [state.tile([P, BS], BF16, name=f"vb{i}") for i in range(ND)]
            w8_s = [state.tile([P, NH, D], FP8, name=f"w8_{k}") for k in range(nk)]
            x0_s = [state.tile([P, BS], F32, name=f"x0_{i}") for i in range(ND)]
            vi0_s = [state.tile([P, BS], BF16, name=f"vi0_{i}") for i in range(ND)]
            vi1_s = [state.tile([P, BS], BF16, name=f"vi1_{i}") for i in range(ND)]
            idb_s = state.tile([P, P], BF16, name="idb")
            xs_s = state.tile([P, ND, BS], F32, name="xs")
            ff_s = [state.tile([P, BS], F32, name=f"ff{i}") for i in range(ND)]
            tdtf_s = [state.tile([P, ND, BS], BF16, name=f"tf{t}")
                      for t in range(steps)]
            if fp8a:
                u8_s = [state.tile([P, 2, H], FP8, name=f"u8_{k}") for k in range(nk)]
                v8_s = state.tile([P, 2, BS], FP8, name="v8")
            else:
                ut_s = [state.tile([P, H], BF16, name=f"ut{i}") for i in range(ND)]

            pv = [ps_state.tile([P, BS], F32, name=f"pv{i}") for i in range(ND)]
            fill_t = (ps_fill.tile([P, P], F32, name="fill")
                      if pe_fill > 0 else None)

            def emit_dmas():
                xfers = []
                if fp8a:
                    xfers.append((v8_s[:, 0, :], v80_d[:, 0:BS]))
                    xfers.append((v8_s[:, 1, :], v80_d[:, BS:2 * BS]))
                else:
                    for i in range(ND):
                        xfers.append((v_bf[i][:], vb0_d[i * P:(i + 1) * P, :]))
                xfers.append((idb_s[:], idb_d[:, :]))
                if fp8a:
                    for k in range(nk):
                        xfers.append((u8_s[k][:], u8_d[k * P:(k + 1) * P, :]))
                else:
                    for i in range(ND):
                        xfers.append((ut_s[i][:], utb_d[i * P:(i + 1) * P, :]))
                for i in range(ND):
                    xfers.append((vi0_s[i][:], vi0_d[i * P:(i + 1) * P, :]))
                    xfers.append((vi1_s[i][:], vi1_d[i * P:(i + 1) * P, :]))
                for k in range(nk):
                    xfers.append((w8_s[k][:], w8_d[k * P:(k + 1) * P, :]))
                for t in range(steps):
                    for i in range(ND):
                        xfers.append((tdtf_s[t][:, i, :],
                                      tdtf_d[(t * ND + i) * P:(t * ND + i + 1) * P, :]))
                for i in range(ND):
                    xfers.append((x0_s[i][:], x0_d[i * P:(i + 1) * P, :]))
                    xfers.append((ff_s[i][:], ff_d[i * P:(i + 1) * P, :]))
                queues = [nc.sync, nc.gpsimd, nc.scalar]
                for qi, (dst, src) in enumerate(xfers):
                    queues[qi % 3].dma_start(dst, src)

            emit_dmas()
            dsq_tiles = None
            if not do_sq:
                dsq_tiles = [state.tile([P, 2, BS], FP8, name=f"dsq{p_}")
                             for p_ in range(NPAIR)]
                for p_ in range(NPAIR):
                    nc.vector.memset(dsq_tiles[p_][:], 0.25)
            nc.vector.memset(xs_s[:], 0.0)
            for i in range(ND):
                nc.tensor.matmul(pv[i][:], idb_s[:], vi0_s[i][:],
                                 start=True, stop=False, skip_group_check=True)
                nc.tensor.matmul(pv[i][:], idb_s[:], vi1_s[i][:],
                                 start=False, stop=True, skip_group_check=True)

            CS = BS // n_streams   # columns per stream

            def emit_xsum(t, s):
                c0, c1 = s * CS, (s + 1) * CS
                # xsum: exact v_t sample from psum_v (reads pv before B(t,s)).
                # t=0 is skippable: v_0 is exact on host (folded into x0pi).
                if do_xsum and not (xsum_skip0 and t == 0) \
                        and not (xsum_wide and s > 0):
                    xc0, xc1 = (0, BS) if xsum_wide else (c0, c1)
                    for i in range(ND):
                        nc.vector.scalar_tensor_tensor(
                            out=xs_s[:, i, xc0:xc1], in0=pv[i][:, xc0:xc1],
                            scalar=float(ALPHA),
                            in1=xs_s[:, i, xc0:xc1], op0=ALU.mult, op1=ALU.add)

            def emit_a_sq(t, s):
                """Phase A + squares for stream s; returns sq group tiles."""
                c0, c1 = s * CS, (s + 1) * CS
                k = t % nk
                r = _r_k(t)

                if fp8a:
                    # scale-free: sqrt(QS/r) folded into u8/v8 host scales,
                    # so sq = uv*uv directly (single-op on either engine)
                    s_act = 1.0
                else:
                    s_act = float(np.sqrt(QS / r))
                # groups of pairs share one uv tile and one ACT square op;
                # sq_groups (e.g. (3,1)) makes the last group small so B's
                # tail waits on a short ACT op
                if sq_groups is not None:
                    grp_sizes = list(sq_groups)
                else:
                    grp_sizes = [sq_span] * (NPAIR // sq_span)
                ngrp = len(grp_sizes)
                grp_base = [sum(grp_sizes[:g]) for g in range(ngrp)]
                sq_grps = []
                for g in range(ngrp):
                    gsz = grp_sizes[g]
                    pool_g = ps_uv if gsz >= max(grp_sizes) else ps_uv2
                    uv_t = pool_g.tile([P, 2 * gsz, CS],
                                       BF16 if uv_bf16 else F32,
                                       tag=f"uv{gsz}", name=f"uv{gsz}")
                    for pp in range(gsz):
                        p_ = grp_base[g] + pp
                        if fp8a:
                            for hh in range(2):
                                ht = 2 * p_ + hh
                                nc.tensor.matmul(
                                    uv_t[:, 2 * pp + hh, :],
                                    u8_s[k][:, :, ht * P:(ht + 1) * P],
                                    v8_s[:, :, c0:c1],
                                    start=True, stop=True, perf_mode=DR,
                                )
                        else:
                            for i in range(ND):
                                for hh in range(2):
                                    ht = 2 * p_ + hh
                                    nc.tensor.matmul(
                                        uv_t[:, 2 * pp + hh, :],
                                        ut_s[i][:, ht * P:(ht + 1) * P],
                                        v_bf[i][:, c0:c1],
                                        start=(i == 0), stop=(i == ND - 1),
                                    )
                    if do_sq:
                        sq_t = sqp.tile([P, 2 * gsz, CS], FP8, tag=f"sq{gsz}",
                                        name="sq")
                        on_dve = (s * ngrp + g) >= ngrp * n_streams - sq_dve_units
                        if not on_dve and sq_sliver > 0 and g == ngrp - 1:
                            # column sliver of each stream's LAST group on DVE:
                            # shortens the B-tail latency (B's final matmuls
                            # wait on a ~half-size ACT op + a parallel DVE op);
                            # scale-free single op: sq = uv*uv
                            cs_ = CS - sq_sliver
                            nc.scalar.activation(sq_t[:, :, 0:cs_],
                                                 uv_t[:, :, 0:cs_],
                                                 ACTF.Square, scale=s_act)
                            nc.vector.tensor_tensor(
                                out=sq_t[:, :, cs_:CS], in0=uv_t[:, :, cs_:CS],
                                in1=uv_t[:, :, cs_:CS], op=ALU.mult)
                        elif not on_dve:
                            nc.scalar.activation(sq_t[:], uv_t[:], ACTF.Square,
                                                 scale=s_act)
                        else:
                            nc.vector.tensor_tensor(
                                out=sq_t[:], in0=uv_t[:], in1=uv_t[:],
                                op=ALU.mult)
                        sq_grps.append(sq_t)
                    else:
                        sq_grps.append(None)

                if pe_fill > 0:
                    # keep the Tensor engine's DVFS p-state hot through the
                    # square-wait gap before phase B (dep-free busy work)
                    for _f in range(pe_fill):
                        nc.tensor.matmul(
                            fill_t[:], idb_s[:], vi0_s[0][:, 0:P],
                            start=True, stop=True, skip_group_check=True)
                return sq_grps, grp_sizes, grp_base

            def emit_b_cast(t, s, sq_pack):
                sq_grps, grp_sizes, grp_base = sq_pack
                c0, c1 = s * CS, (s + 1) * CS
                k = t % nk
                ngrp = len(grp_sizes)
                if not do_b:
                    return
                # interleaved d-half chains; only the last group's matmuls
                # depend on the final square
                for p_ in range(NPAIR):
                    g = max(gi for gi in range(ngrp) if grp_base[gi] <= p_)
                    pp = p_ - grp_base[g]
                    for i in range(ND):
                        if do_sq:
                            rhs = sq_grps[g][:, 2 * pp:2 * pp + 2, :]
                        else:
                            rhs = dsq_tiles[p_][:, :, c0:c1]
                        nc.tensor.matmul(
                            pv[i][:, c0:c1],
                            w8_s[k][:, 2 * p_:2 * p_ + 2, i * P:(i + 1) * P],
                            rhs,
                            start=False, stop=(p_ == NPAIR - 1),
                            perf_mode=DR, skip_group_check=True,
                        )
                # cast for next step's phase A; force ramp re-added here
                # (tdtf_s[t] holds (t+1)*dt*force, pre-scaled by SV for fp8)
                if t + 1 < steps and do_cast:
                    ccols = cast_cols if cast_cols > 0 else CS
                    for cc0 in range(c0, c1, ccols):
                        cce = min(cc0 + ccols, c1)
                        for i in range(ND):
                            if fp8a:
                                nc.vector.scalar_tensor_tensor(
                                    out=v8_s[:, i, cc0:cce],
                                    in0=pv[i][:, cc0:cce],
                                    scalar=float(ALPHA * SV8),
                                    in1=tdtf_s[t][:, i, cc0:cce],
                                    op0=ALU.mult, op1=ALU.add)
                            else:
                                nc.vector.scalar_tensor_tensor(
                                    out=v_bf[i][:, cc0:cce],
                                    in0=pv[i][:, cc0:cce],
                                    scalar=float(ALPHA),
                                    in1=tdtf_s[t][:, i, cc0:cce],
                                    op0=ALU.mult, op1=ALU.add)

            def emit_step(t):
                if phase_major:
                    # hoist every stream's A+squares before any B so the PE
                    # in-order queue never blocks A(s1) behind B(s0)'s
                    # square-waits
                    for s in range(n_streams):
                        emit_xsum(t, s)
                    packs = [emit_a_sq(t, s) for s in range(n_streams)]
                    for s in range(n_streams):
                        emit_b_cast(t, s, packs[s])
                elif xsum_defer:
                    # emit xsum(t+1, s) right after cast(t, s): same spot in
                    # the dep graph (post-B(t,s)), but ahead of the other
                    # stream's cast in the in-order DVE queue
                    for s in range(n_streams):
                        if t == 0:
                            emit_xsum(t, s)
                        emit_b_cast(t, s, emit_a_sq(t, s))
                        if t + 1 < steps:
                            emit_xsum(t + 1, s)
                else:
                    for s in range(n_streams):
                        emit_xsum(t, s)
                        emit_b_cast(t, s, emit_a_sq(t, s))

            loop_cm = (
                tc.For_i(0, loop_reps, 1,
                         staggered_reset=loop_stagger,
                         hint_engines=(mybir.EngineType.PE, mybir.EngineType.DVE,
                                       mybir.EngineType.Activation))
                if loop_reps is not None else contextlib.nullcontext()
            )
            with loop_cm:
                for t in range(steps):
                    emit_step(t)

            # final: vo = alpha*pv + steps*dt*force (exact f32 ramp)
            for i in range(ND):
                vo_t = tmp.tile([P, BS], F32, tag="vo", name="vo")
                nc.vector.scalar_tensor_tensor(
                    out=vo_t[:], in0=pv[i][:], scalar=float(ALPHA),
                    in1=ff_s[i][:], op0=ALU.mult, op1=ALU.add)
                xw = tmp.tile([P, BS], F32, tag="xw", name="xw")
                nc.vector.scalar_tensor_tensor(
                    out=xw[:], in0=xs_s[:, i, :], scalar=float(DT),
                    in1=x0_s[i][:], op0=ALU.mult, op1=ALU.add)
                g = tmp.tile([P, BS], F32, tag="g", name="g")
                nc.vector.tensor_scalar(
                    out=g[:], in0=xw[:], scalar1=TWO_PI, scalar2=None,
                    op0=ALU.is_ge)
                lo = tmp.tile([P, BS], F32, tag="l", name="l")
                nc.vector.tensor_scalar(
                    out=lo[:], in0=xw[:], scalar1=0.0, scalar2=None,
                    op0=ALU.is_lt)
                nc.vector.scalar_tensor_tensor(
                    out=xw[:], in0=g[:], scalar=-TWO_PI, in1=xw[:],
                    op0=ALU.mult, op1=ALU.add)
                nc.vector.scalar_tensor_tensor(
                    out=xw[:], in0=lo[:], scalar=TWO_PI, in1=xw[:],
                    op0=ALU.mult, op1=ALU.add)
                nc.sync.dma_start(xo_d[i * P:(i + 1) * P, :], xw[:])
                nc.scalar.dma_start(vo_d[i * P:(i + 1) * P, :], vo_t[:])

    nc.compile()
    return nc


XSUM_SKIP0 = True
BEST_CONFIG = {"a_mode": "fp8", "n_streams": 2, "sq_span": 2, "uv_bufs": 2,
               "xsum_skip0": XSUM_SKIP0}


def _get_program(steps: int, loop_reps: int | None = None, variant: str = "full",
                 **kw):
    for kk, vv in BEST_CONFIG.items():
        kw.setdefault(kk, vv)
    key = (steps, loop_reps, variant, tuple(sorted(kw.items())))
    if key not in _PROGRAM_CACHE:
        _PROGRAM_CACHE[key] = _build(steps, loop_reps, variant, **kw)
    return _PROGRAM_CACHE[key]


def _make_in_maps(x, v, force, U, W, steps=16, a_mode=None):
    a_mode = a_mode or A_MODE
    fp8a = a_mode == "fp8"
    x = np.asarray(x, np.float32); v = np.asarray(v, np.float32)
    force = np.asarray(force, np.float32)
    U = np.asarray(U, np.float32); W = np.asarray(W, np.float32)

    dtfT = (DT * force.T).astype(np.float32)                # [D,B]
    vT = v.T
    via = (vT.astype(np.float64) / ALPHA)
    vi0 = via.astype(NPBF)
    vi1 = (via - vi0.astype(np.float64)).astype(NPBF)

    wt = (-DT) * W.T.astype(np.float64) * KW                # [H,D]
    nk = min(N_DITHER, steps) if steps > 0 else 1
    w8d = np.zeros((N_DITHER * P, NH * D), NPF8)
    for k in range(nk):
        w8k = _q8(wt * _r_k(k))
        w8d[k * P:(k + 1) * P, :] = w8k.reshape(NH, P, D).transpose(1, 0, 2).reshape(P, NH * D)
    idb = np.eye(P, dtype=np.float32).astype(NPBF)

    base = {"w8d": w8d, "idb": idb}
    # x/v outputs: pv excludes the force ramp in both modes; fold on host
    ramp = float(steps * (steps - 1) / 2)
    x0pi = (x.T + np.float32(PI) + np.float32(ramp * DT) * dtfT).astype(np.float32)
    if fp8a and XSUM_SKIP0 and steps > 0:
        # device skips the t=0 xsum sample; v_0 is exact on host
        x0pi = (x0pi + DT * vT).astype(np.float32)
    ffin = (float(steps) * dtfT).astype(np.float32)
    # tdtf[t] holds (t+1)*dt*force (pre-scaled by SV8 in fp8 mode)
    tsc = SV8 if fp8a else 1.0
    tdtf = np.zeros((steps * D, B), NPBF)
    for t in range(steps):
        for i in range(ND):
            tdtf[(t * ND + i) * P:(t * ND + i + 1) * P, :] = (
                np.float32(tsc * (t + 1)) * dtfT[i * P:(i + 1) * P, :]).astype(NPBF)
    if fp8a:
        # scale-free squares: u8d[k] = q8(U.T * sqrt(QS/r_k) / SV8) so that
        # uv_psum = sqrt(QS/r_k)*uv and sq = uv_psum^2 needs no ACT scale
        utb32 = U.T.astype(np.float64)                       # [D,H]
        u8d = np.zeros((N_DITHER * P, 2 * H), NPF8)
        for k in range(nk):
            su_k = float(np.sqrt(QS / _r_k(k)) / SV8)
            q = _q8(utb32 * su_k)                            # [D,H]
            u8d[k * P:(k + 1) * P, 0:H] = q[0:P, :]
            u8d[k * P:(k + 1) * P, H:2 * H] = q[P:2 * P, :]
    else:
        utb = np.ascontiguousarray(U.T).astype(NPBF)         # [D,H]
        vb0 = vT.astype(NPBF)
        base.update({"utb": utb})

    in_maps = []
    for c in range(N_CORES):
        sl = slice(c * BS, (c + 1) * BS)
        m = dict(base)
        m.update({
            "vi0": np.ascontiguousarray(vi0[:, sl]),
            "vi1": np.ascontiguousarray(vi1[:, sl]),
            "x0pi": np.ascontiguousarray(x0pi[:, sl]),
            "ffin": np.ascontiguousarray(ffin[:, sl]),
        })
        m["tdtf"] = np.ascontiguousarray(tdtf[:, sl])
        if fp8a:
            m["u8d"] = u8d
            m["v80"] = np.ascontiguousarray(
                np.concatenate([_q8(vT[0:P, sl] * SV8), _q8(vT[P:2 * P, sl] * SV8)], axis=1))
        else:
            m["vb0"] = np.ascontiguousarray(vb0[:, sl])
        in_maps.append(m)
    return in_maps


def _run(x, v, force, U, W, steps):
    steps = int(np.asarray(steps).item()) if not isinstance(steps, int) else steps
    if steps == 0:
        return (np.asarray(x, np.float32).copy(),
                np.asarray(v, np.float32).copy())
    nc = _get_program(steps)
    in_maps = _make_in_maps(x, v, force, U, W, steps)
    res = run_bass_kernel_spmd(nc, in_maps, list(range(N_CORES)))
    xo = np.concatenate([res.results[c]["xo"].T for c in range(N_CORES)], axis=0)
    vo = np.concatenate([res.results[c]["vo"].T for c in range(N_CORES)], axis=0)
    xo = (xo - np.float32(PI)).astype(np.float32)
    return xo, vo


def kernel(x, v, force, U, W, steps):
    return _run(x, v, force, U, W, steps)

